# revision 21
# baseline (speedup 1.0000x reference)
"""Trainium2 Bass kernel for the NAS LSTM controller rollout (nn_Controller).

Strategy (8 NeuronCores, tensor-parallel on the 4H gate dim):
  - The recurrence h_{t+1} = LSTM(h_t, x_t) is sequential; x_{t+1} =
    encoder[min(op_t+1, 11)] where op_t is a categorical sample.
  - W_ih is algebraically eliminated: gates_ih for every possible op is
    precomputed on the host as Wx[c] = encoder[min(c+1,11)] @ W_ih.T + b_ih
    + b_hh (a [12, 8192] table); on device it is a one-hot [12] x [12,1024]
    matmul per core.  Only W_hh (64 MB) streams through the PE each step.
  - W_hh is sharded 8 ways on the gate dim (each core owns 256 h-dims of
    each of the i/f/o/g gate blocks = [1024, 2048] rows), SBUF-resident.
  - Per step each core computes its 256-dim shard of c/h, AllGathers h
    (1 KB per rank), then every core redundantly runs the tiny sampling
    head (w_soft matvec, tanh, precomputed-Gumbel argmax) so no further
    communication is needed.
  - jax.random.categorical(key, logits) == argmax(logits + gumbel(key));
    the 32x12 Gumbel noise for key(42) is input-independent and baked in.
  - nll/ent use per-step tanh-logit columns stored in SBUF [12, 32] and
    are reduced in a handful of batched ops after the loop.
"""

import base64
import sys

sys.path.insert(0, "/opt/trn_rl_repo")

import numpy as np

import concourse.bacc as bacc
import concourse.mybir as mybir
import concourse.tile as tile
from concourse import bass_utils

NCORES = 8
H = 2048
C = 12
T = 32
SH = H // NCORES            # h-dims per core (256)
Q = SH // 4                 # h-dims per col-group quarter (64)
KF = 16                     # contraction split: k = p*16 + f, p in [0,128)
TEMP = 5.0
TANH_C = 2.5
# within each psum row the free dim holds [i | f | o | g] (64 each), i.e.
# gate order [i, f, o, g] = torch gate types [0, 1, 3, 2].  Col-group j
# computes h-quarter j of all four gates so every elementwise op in the
# LSTM cell stays partition-aligned (DVE/ACT lanes cannot cross partitions).
GATE_ORDER = [0, 1, 3, 2]

_GUMBEL_B64 = (
    "02sgPmIVP0Cc3p0+DhblvmInA0DIOOs/LwnMPvySOD5EIEy/GkGxvl4ExD6mWkU/mQAxQDowGr+xvR+/"
    "hwZ4vmhJKT/YZfk/EQbiPcpxWUBK2+4/I2bJP5odJz9mDMM/KWWDPx752738GI6/yHirvrQrYj15fwS8"
    "kqTnvrK0Db/ErztAeNkOv0ukgD8GSLy9B+/SPkHplD+Cy50/an+4v0ZGmj+IgZg/7I1zvmYfnr88bBs/"
    "JpijP/+el77CPQM/WJmtvYbaWD+xtL2/FOCgQK3PQb3YHgw/X38PPuh2ET/E1dk/KeWRPmKFkD6V3V6+"
    "GloAPw0U1z91pAlAxHs3vnoQ0r6o7rw+HQrRvq5HOD9IxR8+9ohsP5a1KD8kn2S/5hhtPwUPCT+gris/"
    "Crw2P58ctT4KVLq+p45aPb8pib6dUJE/4oirPq4BbL5sH5s/kYJBP5B4gD8Nxhe9jua3PwC+8b68Cz+/"
    "UDMGP4J3BkAU8d2+zMtBPpOIUUC/JkQ/Lcc7P9ASQEBOWatAJN86vNIMrr6j2WW+/tdgQIQD2D4fSpu9"
    "Kur+P/bTw7+2EdM+uxsIP1o0KUAeEXc/sJ5Ev9uICr59Wo+92eBNvrcd0j8+XTo/vxYhPzFUIL9E1kK+"
    "demnv/SUNL+1jl0++mTOvr2yqT5TDk48sHVKPwn2mj6iAJ2/Y6p5vxcKrT+AZVe+0pS1Pl7sKz+6HSm+"
    "ZWNpPy46XD8lq8g9HjHVvgi7sD9E8t0/GgqzP+F+Mj8oRmk7q2zaP3jfkD6plEC/9gSrPmBpB77jC7c/"
    "N1KLvo2BYkCfyFpA1t94PfJWiz1kPau9D4oCQHJfCz9nN3m+KhbuP6DAuj/G9Ge/NjSdPw8dNj+LzAC+"
    "BBMovyIM0D+qoFE/5bk7vtHJpT++BeY/JHAoP3L2QD9lxS0/NDTDPV50RL9lmLg9eFgbv0mh1r4DyaY+"
    "lhsTv9BHWT9UOos+j2p+QJrEXD/h2IU/NJk/v0TRnL9N6qU/G9TyPUeI0b5mmEQ/dARcP4ajiT9AUy2/"
    "irciP025VEANK14/9I+cP7aWV7+oB9w+Kh7TvkBYlT9Qt0e/a45CQKqmoj41bD2/zeAdPzf1CkDCgiM/"
    "9HB8P8GlWT/C2QK/iK08QFjsg78fPTO/B2r0PjTZJL+3tGG/V28jP+4pXT6Y2AM/6typPnTfhD8t4k89"
    "CTcMQOxi8r7MeZZAEKohvtcrzr2o2I4/TFqGPyg+Rb4k2H0/aV0SQIhkTkBAugE+Nii3QGu9gL+1WZm/"
    "+AOKP6WcpD9graE/azWMvlaaAUAAn1g/3RevP8tDkb77aIY/ICRnP6ElUz6Akak+hvcZv41rO74/kMo9"
    "NGbOPr2LGD9qKcA/rAwlPs8sjD9xcr4/ARlhQBxt8T4sBqC/FcUMv0pVlz80j+I+wM9iP+3G/z9e/JA/"
    "bhMuPWqaxT/0I2m/iKSNPzh9uD/eom89IPETv9dy2T+UDEdAAPAbPia40z7ZLFG/HHQhv2m13D8rF8M+"
    "riVCQClGgT/uv8y97s3yPn3qDz+BybQ/lrxiPhvBoD4njL0+PpOiP/IzTj8jhb6/PqQgQBPxmj5KD7xA"
    "AsPgvsDtYj+77kC/BVkKv9awAr9Mm4w/LdChvHhdkT+0TkW/hFc/P7YioT9HnRRA4iSWv9XygL/By4e/"
    "FkdBvhIupz/iUX4+Y9uNPsT12D9F/C4/dOozvxPr9b254TC/n/RBPZXdu75sDcM/Z7DOPwycVr+EfhW/"
    "Xr8Lv/4B1b2wV3w/ckGFQEbPkL/VAoY/ZtwNQMKdmT+c5ju+7HlYPvgSBr9Dd08+y4xFQIFxRz/0pjO+"
    "4n6UPwwTa7/dTDW/vmxHQIgfOj/Zx34/K7YAvv6/AkBT0MC9plk7vxBuyr5PSkg+YZQDQN3nuj9dF0a9"
    "gkY7v+2mNjyCUlk/w1UGQNcBUkB1oC2/LHdUv19/db50Z6g//DHpP2gTDj7fO/s/u+ASPg0Gkz1vg12+"
    "fgMyPmOhxL96kbW+2xmuPziXjT8Ryc8/Rm0KQI67uD/88Iy+"
)
GUMBEL = np.frombuffer(base64.b64decode(_GUMBEL_B64), np.float32).reshape(T, C)

_CACHE = {}


def _shard_rows(r):
    """Global W rows (torch i,f,g,o order) owned by core r, in device
    output order: quarter-major, [i_q | f_q | o_q | g_q] x 64 per quarter."""
    rows = []
    for q in range(4):
        for g in GATE_ORDER:
            base = g * H + r * SH + q * Q
            rows.extend(range(base, base + Q))
    return np.array(rows)


def _build_program():
    nc = bacc.Bacc(
        "TRN2",
        target_bir_lowering=False,
        debug=False,
        enable_asserts=True,
        num_devices=NCORES,
    )
    dt = mybir.dt.float32

    whh_d = nc.dram_tensor("whh", [128, KF * 4 * SH], dt, kind="ExternalInput")
    wx_d = nc.dram_tensor("wx", [C, 4 * SH], dt, kind="ExternalInput")
    g0_d = nc.dram_tensor("g0", [4, SH], dt, kind="ExternalInput")
    cinit_d = nc.dram_tensor("cinit", [1, SH], dt, kind="ExternalInput")
    wsoft_d = nc.dram_tensor("wsoft", [128, KF * C], dt, kind="ExternalInput")
    gum_d = nc.dram_tensor("gum", [C, T], dt, kind="ExternalInput")
    eye_d = nc.dram_tensor("eye12", [C, C], dt, kind="ExternalInput")
    onesr_d = nc.dram_tensor("ones_r", [1, C], dt, kind="ExternalInput")
    onesc_d = nc.dram_tensor("ones_c", [C, 1], dt, kind="ExternalInput")
    iota_d = nc.dram_tensor("iota12", [C, 1], dt, kind="ExternalInput")

    ops_d = nc.dram_tensor("op_seq", [1, T], mybir.dt.int32, kind="ExternalOutput")
    nll_d = nc.dram_tensor("nll", [1, 1], dt, kind="ExternalOutput")
    ent_d = nc.dram_tensor("ent", [1, 1], dt, kind="ExternalOutput")

    AF = mybir.ActivationFunctionType
    OP = mybir.AluOpType

    with tile.TileContext(nc) as tc:
        with (
            tc.tile_pool(name="const", bufs=1) as cp,
            tc.tile_pool(name="state", bufs=1) as sp,
            tc.tile_pool(name="psum", bufs=2, space="PSUM") as pp,
            tc.tile_pool(name="psum1", bufs=1, space="PSUM") as pp1,
            tc.tile_pool(name="dram", bufs=1, space="DRAM") as dp,
        ):
            whh_t = cp.tile([128, KF * 4 * SH], dt, tag="whh")
            wx_t = cp.tile([C, 4 * SH], dt, tag="wx")
            wsoft_t = cp.tile([128, KF * C], dt, tag="wsoft")
            gum_t = cp.tile([C, T], dt, tag="gum")
            eye_t = cp.tile([C, C], dt, tag="eye")
            ones_r = cp.tile([1, C], dt, tag="onesr")
            ones_c = cp.tile([C, 1], dt, tag="onesc")
            iota_t = cp.tile([C, 1], dt, tag="iota")

            h_tile = sp.tile([128, KF], dt, tag="h_tile")
            c_st = sp.tile([1, SH], dt, tag="c_st")
            h2_st = sp.tile([1, SH], dt, tag="h2_st")
            t_buf = sp.tile([C, T], dt, tag="t_buf")
            mask_buf = sp.tile([C, T], dt, tag="mask_buf")
            g0_t = sp.tile([128, SH], dt, tag="g0")
            # row 0 free layout: [i(256) | f(256) | o(256) | tanh(g)(256)]
            lstm_sb = sp.tile([1, 4 * SH], dt, tag="lstm")
            tmp_a = sp.tile([1, SH], dt, tag="tmp_a")
            tmp_b = sp.tile([1, SH], dt, tag="tmp_b")
            v_sb = sp.tile([C, 1], dt, tag="v_sb")
            m_sb = sp.tile([1, 1], dt, tag="m_sb")
            c0_t = sp.tile([1, SH], dt, tag="c0")

            # ---- constant / weight loads ----
            for f in range(KF):
                nc.sync.dma_start(
                    whh_t[:, f * 1024:(f + 1) * 1024],
                    whh_d[:, f * 1024:(f + 1) * 1024],
                )
            nc.sync.dma_start(wx_t[:], wx_d[:])
            nc.sync.dma_start(wsoft_t[:], wsoft_d[:])
            nc.sync.dma_start(gum_t[:], gum_d[:])
            nc.sync.dma_start(eye_t[:], eye_d[:])
            nc.sync.dma_start(ones_r[:], onesr_d[:])
            nc.sync.dma_start(ones_c[:], onesc_d[:])
            nc.sync.dma_start(iota_t[:], iota_d[:])
            for j in range(4):
                nc.sync.dma_start(g0_t[32 * j:32 * j + 1, :], g0_d[j:j + 1, :])
            nc.sync.dma_start(c0_t[:], cinit_d[:])

            # strided view of lstm_sb row 0 for the quarter-consolidating
            # activations: [1, gate(3), quarter(4), 64]
            lstm_sig = lstm_sb[:, 0:3 * SH].rearrange(
                "p (a q u) -> p a q u", a=3, q=4, u=Q
            )

            def epilogue(gates, c_prev):
                """gates: [128, SH] AP, rows {0,32,64,96} = h-quarters, free
                [i|f|o|g]x64.  Consolidates to lstm_sb row 0 and computes
                c_st / h2_st ([1, 256])."""
                for j in range(4):
                    nc.scalar.activation(
                        lstm_sig[:, :, j, :],
                        gates[32 * j:32 * j + 1, 0:3 * Q].rearrange(
                            "p (a u) -> p a u", a=3, u=Q
                        ),
                        AF.Sigmoid,
                    )
                    nc.scalar.activation(
                        lstm_sb[:, 3 * SH + j * Q:3 * SH + (j + 1) * Q],
                        gates[32 * j:32 * j + 1, 3 * Q:4 * Q],
                        AF.Tanh,
                    )
                nc.vector.tensor_mul(
                    tmp_a[:], lstm_sb[:, 0:SH], lstm_sb[:, 3 * SH:4 * SH]
                )
                nc.vector.tensor_mul(tmp_b[:], lstm_sb[:, SH:2 * SH], c_prev)
                nc.vector.tensor_add(c_st[:], tmp_a[:], tmp_b[:])
                nc.scalar.activation(tmp_a[:], c_st[:], AF.Tanh)
                nc.vector.tensor_mul(
                    h2_st[:], lstm_sb[:, 2 * SH:3 * SH], tmp_a[:]
                )

            # step 0 epilogue: gates = g0 (includes prev_h @ W_hh.T + b), c = cinit
            epilogue(g0_t[:], c0_t[:])

            for t in range(T):
                ag_in = dp.tile([1, SH], dt, tag=f"agin{t}")
                ag_out = dp.tile([128, KF], dt, tag=f"agout{t}")
                nc.sync.dma_start(ag_in[:], h2_st[:])
                nc.gpsimd.collective_compute(
                    "AllGather",
                    OP.bypass,
                    ins=[ag_in[:].opt()],
                    outs=[ag_out[:].opt()],
                    replica_groups=[list(range(NCORES))],
                )
                nc.sync.dma_start(h_tile[:], ag_out[:])

                # logits^T [12, 1] = (w_soft/TEMP) @ h
                lg = pp1.tile([C, 1], dt, tag="lgT")
                for f in range(KF):
                    nc.tensor.matmul(
                        lg[:],
                        wsoft_t[:, f * C:(f + 1) * C],
                        h_tile[:, f:f + 1],
                        start=(f == 0),
                        stop=(f == KF - 1),
                    )
                # sampling: t_col = tanh(lg); v = t_col + gumbel'/2.5; argmax
                nc.scalar.activation(t_buf[:, t:t + 1], lg[:], AF.Tanh)
                nc.vector.tensor_add(v_sb[:], t_buf[:, t:t + 1], gum_t[:, t:t + 1])
                vT = pp1.tile([1, C], dt, tag="vT")
                nc.tensor.transpose(vT[:], v_sb[:], eye_t[:])
                nc.vector.reduce_max(m_sb[:], vT[:], axis=mybir.AxisListType.X)
                mb = pp1.tile([C, 1], dt, tag="mb")
                nc.tensor.matmul(mb[:], ones_r[:], m_sb[:], start=True, stop=True)
                nc.vector.tensor_tensor(
                    mask_buf[:, t:t + 1], v_sb[:], mb[:], op=OP.is_equal
                )

                if t < T - 1:
                    gps = pp.tile([128, SH], dt, tag="gates")
                    for f in range(KF):
                        for j in range(4):
                            nc.tensor.matmul(
                                gps[32 * j:32 * j + 1, :],
                                h_tile[:, f:f + 1],
                                whh_t[:, f * 1024 + j * SH:f * 1024 + (j + 1) * SH],
                                tile_position=(0, 32 * j),
                                start=(f == 0),
                                stop=False,
                            )
                    for j in range(4):
                        nc.tensor.matmul(
                            gps[32 * j:32 * j + 1, :],
                            mask_buf[:, t:t + 1],
                            wx_t[:, j * SH:(j + 1) * SH],
                            tile_position=(0, 32 * j),
                            start=False,
                            stop=True,
                        )
                    epilogue(gps[:], c_st[:])

            # ---- end phase: batched nll / ent / op extraction ----
            e_t = sp.tile([C, T], dt, tag="e_t")
            tm = sp.tile([C, T], dt, tag="tm")
            logS = sp.tile([1, T], dt, tag="logS")
            sinv = sp.tile([1, T], dt, tag="sinv")
            row_a = sp.tile([1, T], dt, tag="row_a")
            row_b = sp.tile([1, T], dt, tag="row_b")
            nll_sb = sp.tile([1, 1], dt, tag="nll_sb")
            ent_sb = sp.tile([1, 1], dt, tag="ent_sb")
            ops_i = sp.tile([1, T], mybir.dt.int32, tag="ops_i")

            S_ps = pp1.tile([1, T], dt, tag="lgT")
            nc.scalar.activation(e_t[:], t_buf[:], AF.Exp, scale=TANH_C)
            nc.tensor.matmul(S_ps[:], ones_c[:], e_t[:], start=True, stop=True)
            nc.scalar.activation(logS[:], S_ps[:], AF.Ln)
            nc.vector.reciprocal(sinv[:], S_ps[:])

            tsel_ps = pp1.tile([1, T], dt, tag="vT")
            nc.vector.tensor_mul(tm[:], t_buf[:], mask_buf[:])
            nc.tensor.matmul(tsel_ps[:], ones_c[:], tm[:], start=True, stop=True)
            # nll_row = logS - TANH_C * tsel ; nll = sum(nll_row)
            nc.vector.scalar_tensor_tensor(
                row_a[:], tsel_ps[:], -TANH_C, logS[:],
                op0=OP.mult, op1=OP.add, accum_out=nll_sb[:],
            )

            et_ps = pp1.tile([1, T], dt, tag="mb")
            nc.vector.tensor_mul(tm[:], e_t[:], t_buf[:])
            nc.tensor.matmul(et_ps[:], ones_c[:], tm[:], start=True, stop=True)
            nc.vector.tensor_mul(row_b[:], et_ps[:], sinv[:])
            nc.vector.scalar_tensor_tensor(
                row_b[:], row_b[:], -TANH_C, logS[:],
                op0=OP.mult, op1=OP.add, accum_out=ent_sb[:],
            )

            ops_ps = pp.tile([1, T], dt, tag="gates")
            nc.tensor.matmul(ops_ps[:], iota_t[:], mask_buf[:], start=True, stop=True)
            nc.vector.tensor_copy(ops_i[:], ops_ps[:])

            nc.sync.dma_start(ops_d[:], ops_i[:])
            nc.sync.dma_start(nll_d[:], nll_sb[:])
            nc.sync.dma_start(ent_d[:], ent_sb[:])

    nc.compile()
    return nc


def _prepare_inputs(encoder, w_ih, w_hh, b_ih, b_hh, w_soft, prev_c, prev_h):
    f64 = np.float64
    enc = np.asarray(encoder, f64)
    wih = np.asarray(w_ih, f64)
    whh = np.asarray(w_hh, f64)
    b = np.asarray(b_ih, f64) + np.asarray(b_hh, f64)
    ph = np.asarray(prev_h, f64).reshape(H)
    pc = np.asarray(prev_c, np.float32).reshape(H)

    # fused next-x gate table and step-0 gates (include prev_h recurrence)
    wx_full = np.stack([enc[min(c + 1, C - 1)] @ wih.T + b for c in range(C)])
    g0_full = enc[0] @ wih.T + b + ph @ whh.T

    ws = (np.asarray(w_soft, f64) / TEMP).astype(np.float32)
    # wsoft layout: [p, f*12 + c] = ws[c, p*16 + f]
    wsoft_l = np.ascontiguousarray(
        ws.T.reshape(128, KF, C).reshape(128, KF * C)
    ).astype(np.float32)

    gum = np.ascontiguousarray((GUMBEL / TANH_C).T).astype(np.float32)  # [C, T]
    eye = np.eye(C, dtype=np.float32)
    ones_r = np.ones((1, C), np.float32)
    ones_c = np.ones((C, 1), np.float32)
    iota = np.arange(C, dtype=np.float32).reshape(C, 1)

    whh32 = whh.astype(np.float32)
    in_maps = []
    for r in range(NCORES):
        rows = _shard_rows(r)
        wsh = whh32[rows]                      # [1024, 2048]
        # whh layout: [p, f*1024 + n] = wsh[n, p*16 + f]
        whh_l = np.ascontiguousarray(
            wsh.T.reshape(128, KF, 4 * SH).reshape(128, KF * 4 * SH)
        ).astype(np.float32)
        wx_l = np.ascontiguousarray(wx_full[:, rows]).astype(np.float32)
        g0_l = np.ascontiguousarray(g0_full[rows]).reshape(4, SH).astype(np.float32)
        c_l = np.ascontiguousarray(pc[r * SH:(r + 1) * SH]).reshape(1, SH)
        in_maps.append({
            "whh": whh_l,
            "wx": wx_l,
            "g0": g0_l,
            "cinit": c_l,
            "wsoft": wsoft_l,
            "gum": gum,
            "eye12": eye,
            "ones_r": ones_r,
            "ones_c": ones_c,
            "iota12": iota,
        })
    return in_maps


def kernel(prev_c, prev_h, encoder, w_ih, w_hh, b_ih, b_hh, w_soft):
    if "nc" not in _CACHE:
        _CACHE["nc"] = _build_program()
    nc = _CACHE["nc"]
    in_maps = _prepare_inputs(encoder, w_ih, w_hh, b_ih, b_hh, w_soft,
                              prev_c, prev_h)
    _CACHE["in_maps"] = in_maps
    res = bass_utils.run_bass_kernel_spmd(
        nc, in_maps, core_ids=list(range(NCORES))
    )
    out = res.results[0]
    op_seq = np.asarray(out["op_seq"], np.int32).reshape(T)
    nll = np.float32(np.asarray(out["nll"]).reshape(())[()])
    ent = np.float32(np.asarray(out["ent"]).reshape(())[()])
    return op_seq, nll, ent


def measure_exec_ns(reps=24):
    """Estimate device execution time per kernel run by timing a pipeline of
    async PJRT dispatches with device-resident inputs (the host round-trip
    overhead amortizes across the queue; no NTFF profiling exists under this
    axon client).  Returns ns per run from the (reps vs 2-rep) slope."""
    import time

    import jax
    from jax.sharding import Mesh, PartitionSpec
    from jax.experimental.shard_map import shard_map

    from concourse import bass2jax, mybir as _mb

    nc = _CACHE["nc"]
    in_maps = _CACHE["in_maps"]
    bass2jax.install_neuronx_cc_hook()

    part_name = nc.partition_id_tensor.name if nc.partition_id_tensor else None
    in_names, out_names, out_avals, zero_outs = [], [], [], []
    for alloc in nc.m.functions[0].allocations:
        if not isinstance(alloc, _mb.MemoryLocationSet):
            continue
        name = alloc.memorylocations[0].name
        if alloc.kind == "ExternalInput":
            if name != part_name:
                in_names.append(name)
        elif alloc.kind == "ExternalOutput":
            shape = tuple(alloc.tensor_shape)
            dtype = _mb.dt.np(alloc.dtype)
            out_names.append(name)
            out_avals.append(jax.core.ShapedArray(shape, dtype))
            zero_outs.append(np.zeros(shape, dtype))
    n_params = len(in_names)
    all_names = in_names + out_names
    if part_name is not None:
        all_names = all_names + [part_name]

    def _body(*args):
        operands = list(args)
        if part_name is not None:
            operands.append(bass2jax.partition_id_tensor())
        return tuple(bass2jax._bass_exec_p.bind(
            *operands,
            out_avals=tuple(out_avals),
            in_names=tuple(all_names),
            out_names=tuple(out_names),
            lowering_input_output_aliases=(),
            sim_require_finite=True,
            sim_require_nnan=True,
            nc=nc,
        ))

    devices = jax.devices()[:NCORES]
    mesh = Mesh(np.asarray(devices), ("core",))
    fn = jax.jit(
        shard_map(
            _body, mesh=mesh,
            in_specs=(PartitionSpec("core"),) * (n_params + len(out_names)),
            out_specs=(PartitionSpec("core"),) * len(out_names),
            check_rep=False,
        ),
        keep_unused=True,
    )
    concat_in = [
        np.concatenate([in_maps[c][n] for c in range(NCORES)], axis=0)
        for n in in_names
    ]
    concat_zeros = [
        np.zeros((NCORES * z.shape[0], *z.shape[1:]), z.dtype) for z in zero_outs
    ]
    sharding = jax.sharding.NamedSharding(mesh, PartitionSpec("core"))
    dev_in = [jax.device_put(a, sharding) for a in concat_in]
    dev_zero = [jax.device_put(a, sharding) for a in concat_zeros]

    def run_n(n):
        outs = None
        t0 = time.perf_counter()
        for _ in range(n):
            outs = fn(*dev_in, *dev_zero)
        for o in outs:
            o.block_until_ready()
        return time.perf_counter() - t0

    run_n(3)  # warm
    lo = min(run_n(2) for _ in range(3))
    hi = min(run_n(2 + reps) for _ in range(3))
    return max(0, int((hi - lo) / reps * 1e9))
